# revision 21
# baseline (speedup 1.0000x reference)
"""Distributed GIN (3-layer, BatchNorm, sum-pool, MLP classifier) on 8 TRN2 NeuronCores.

Strategy (node sharding, per sharding_hint):
  - Nodes are sharded contiguously across 8 cores (12500 nodes/core).
  - Each core owns the edges whose dst lands in its shard (plus self-loops,
    which implement the (1+eps)*h term with eps=0).
  - Host pre-sorts edges by (dst tile, src) and packs them into a fixed
    [128 partitions x T*C columns] slot layout; every 128-slot chunk belongs to
    one 128-node dst tile.
  - Per layer: every core holds a full replica of h ([100000,128]); it gathers
    h[src] rows with one indirect DMA per dst tile, reduces segments with
    one-hot matmuls on the TensorEngine (S built on-the-fly on the
    VectorEngine), applies the MLP with BatchNorm whose global stats come from
    a tiny AllReduce, and publishes its updated 12500-row shard with an
    AllGather.
  - Pooling: batch is sorted, so each core's nodes map to <=256 graphs;
    one-hot matmul pooling into a 256-graph window, indirect-scatter into a
    [1024,128] buffer, AllReduce, then every core redundantly computes the
    tiny classifier and writes the full [1024,64] output.
"""

import os
import sys

import numpy as np

sys.path.insert(0, "/opt/trn_rl_repo")

# ---- problem constants (hardcoded per contract) ----
N = 100_000
F = 128
NCORES = 8
NSHARD = N // NCORES          # 12500
TILE = 128
T = (NSHARD + TILE - 1) // TILE   # 98 tiles/core; last tile has 84 nodes
LAST_ROWS = NSHARD - (T - 1) * TILE  # 84
NG = 1024
NOUT = 64
L = 3
BN_EPS = 1e-5
PADVAL = 1000.0
GWIN = 256                    # per-core graph window for pooling
GROUP = 7 if os.environ.get("GIN_DT", "bfloat16") == "bfloat16" else 2
NGRP = T // GROUP             # gather groups (98 = 14*7 = 49*2)

DT_STR = os.environ.get("GIN_DT", "bfloat16")


# ======================================================================
# host-side preprocessing
# ======================================================================

def _pack_nodes(deg, nnodes):
    """Greedy best-fit packing of one core's nodes into T tiles.

    deg: [nnodes, 4] per-src-chunk edge counts (incl. self-loop).
    Returns (tile, pos) per node and the chunk capacity C4 used.
    """
    for C4 in (5, 6, 7, 8):
        cap = C4 * TILE
        loads = np.zeros((T, 4), np.int64)
        counts = np.zeros(T, np.int64)
        ncap = np.full(T, TILE, np.int64)
        ncap[T - 1] = LAST_ROWS
        order = np.argsort(-deg.sum(1), kind="stable")
        tile_of = np.full(nnodes, -1, np.int64)
        pos_of = np.zeros(nnodes, np.int64)
        ok = True
        for n in order:
            valid = (counts < ncap) & np.all(loads + deg[n] <= cap, axis=1)
            if not valid.any():
                ok = False
                break
            # best fit: most remaining node slots
            b = int(np.argmax(np.where(valid, ncap - counts, -1)))
            tile_of[n] = b
            pos_of[n] = counts[b]
            counts[b] += 1
            loads[b] += deg[n]
        if ok:
            return tile_of, pos_of, C4
    raise RuntimeError("node packing failed")


def _prep_edges(edge_index):
    """Returns (perm, iperm, idx16, dloc, C4).

    perm[new_id] = old_id (within-core permutation); tables are stored in
    new-id order.  idx16: [NCORES, 128, ICOL] int16 gather indices (wrapped in
    16 partitions, replicated x8).  dloc: [NCORES, 128, QCOL] f32 one-hot
    columns.  Chunk c4 of a source = old_core_pair = new_id // 25000.
    """
    src = np.asarray(edge_index[0]).astype(np.int64)
    dst = np.asarray(edge_index[1]).astype(np.int64)
    loops = np.arange(N, dtype=np.int64)
    src_all = np.concatenate([src, loops])
    dst_all = np.concatenate([dst, loops])
    c4e = src_all // 25000

    deg = np.zeros((N, 4), np.int64)
    np.add.at(deg, (dst_all, c4e), 1)

    perm = np.zeros(N, np.int64)
    C4 = 0
    for c in range(NCORES):
        dl_ = deg[c * NSHARD:(c + 1) * NSHARD]
        tile_of, pos_of, C4c = _pack_nodes(dl_, NSHARD)
        C4 = max(C4, C4c)
        newid = c * NSHARD + tile_of * TILE + pos_of
        perm[newid] = c * NSHARD + np.arange(NSHARD)
    iperm = np.zeros(N, np.int64)
    iperm[perm] = np.arange(N)

    dst_new = iperm[dst_all]
    src_new = iperm[src_all]
    core = dst_new // NSHARD
    off = dst_new - core * NSHARD
    t = off // TILE
    dst_local = off - t * TILE
    g = t // GROUP
    ti = t - g * GROUP
    c4 = src_new // 25000
    idx16v = (src_new - c4 * 25000).astype(np.int16)

    gk = ((core * NGRP + g) * 4 + c4) * GROUP + ti
    order = np.lexsort((src_new, gk))
    gk_s = gk[order]
    idx_s = idx16v[order]
    dl_s = dst_local[order]

    nkeys = NCORES * NGRP * 4 * GROUP
    cnt = np.bincount(gk_s, minlength=nkeys)
    assert cnt.max() <= C4 * TILE, (cnt.max(), C4 * TILE)
    starts = np.zeros(nkeys, np.int64)
    starts[1:] = np.cumsum(cnt)[:-1]
    rank = np.arange(len(gk_s)) - starts[gk_s]
    j = rank // TILE
    p = rank % TILE

    cc = gk_s // (NGRP * 4 * GROUP)
    rem = gk_s - cc * (NGRP * 4 * GROUP)
    gg = rem // (4 * GROUP)
    rem2 = rem - gg * (4 * GROUP)
    c4s = rem2 // GROUP
    tis = rem2 - c4s * GROUP

    NI = GROUP * C4 * TILE               # idxs per gather call
    ICOL = NGRP * 4 * (NI // 16)
    QCOL = NGRP * 4 * GROUP * C4

    # gather-index array (16-partition wrapped)
    i_flat = (tis * C4 + j) * TILE + p   # position within call
    basecol = (gg * 4 + c4s) * (NI // 16)
    idx16 = np.zeros((NCORES, 16, ICOL), np.int16)
    idx16[cc, i_flat % 16, basecol + i_flat // 16] = idx_s
    idx16 = np.tile(idx16, (1, 8, 1))    # replicate to 128 partitions

    # one-hot dst_local columns
    q = gg * (4 * GROUP * C4) + (c4s * GROUP + tis) * C4 + j
    dloc = np.full((NCORES, TILE, QCOL), PADVAL, np.float32)
    dloc[cc, p, q] = dl_s
    return perm, idx16, dloc, C4


def _prep_batch(batch, perm):
    b = np.asarray(batch).astype(np.int64)
    bl = np.full((NCORES, TILE, T), PADVAL, np.float32)
    grows = np.zeros((NCORES, TILE, 2), np.int32)
    for c in range(NCORES):
        seg = b[perm[c * NSHARD:(c + 1) * NSHARD]]
        g0 = int(seg.min())
        span = int(seg.max()) - g0 + 1
        assert span <= GWIN, f"graph window {span} > {GWIN}"
        loc = (seg - g0).astype(np.float32)
        pad = np.full(T * TILE - NSHARD, PADVAL, np.float32)
        bl[c] = np.concatenate([loc, pad]).reshape(T, TILE).T
        grows[c] = (g0 + np.arange(GWIN, dtype=np.int32)).reshape(2, TILE).T
    return bl, grows


# ======================================================================
# bass kernel builder
# ======================================================================

def _build(C4, np_dt, my_dt):
    import concourse.bacc as bacc
    import concourse.tile as tile
    from concourse import bass, mybir
    from concourse.masks import make_identity
    from contextlib import ExitStack

    f32 = mybir.dt.float32
    i32 = mybir.dt.int32
    i16 = mybir.dt.int16
    NI = GROUP * C4 * TILE
    ICOL = NGRP * 4 * (NI // 16)
    QCOL = NGRP * 4 * GROUP * C4
    GCOLS = 4 * GROUP * C4               # G-tile chunk columns per group
    AX = mybir.AxisListType
    OP = mybir.AluOpType
    AF = mybir.ActivationFunctionType

    nc = bacc.Bacc("TRN2", target_bir_lowering=False, debug=False,
                   num_devices=NCORES)
    rg = [list(range(NCORES))]

    # ---- I/O ----
    xt = nc.dram_tensor("x_table", [N, F], my_dt, kind="ExternalInput")
    idx_d = nc.dram_tensor("idx", [TILE, ICOL], i16, kind="ExternalInput")
    dloc_d = nc.dram_tensor("dloc", [TILE, QCOL], f32, kind="ExternalInput")
    bl_d = nc.dram_tensor("blocal", [TILE, T], f32, kind="ExternalInput")
    grows_d = nc.dram_tensor("grows", [TILE, 2], i32, kind="ExternalInput")
    w1_d, w2_d, gam_d, bet_d, b2_d = [], [], [], [], []
    for l in range(L):
        w1_d.append(nc.dram_tensor(f"w1_{l}", [F, F], my_dt, kind="ExternalInput"))
        w2_d.append(nc.dram_tensor(f"w2_{l}", [F, F], my_dt, kind="ExternalInput"))
        gam_d.append(nc.dram_tensor(f"gam_{l}", [F, 1], f32, kind="ExternalInput"))
        bet_d.append(nc.dram_tensor(f"bet_{l}", [F, 1], f32, kind="ExternalInput"))
        b2_d.append(nc.dram_tensor(f"b2_{l}", [F, 1], f32, kind="ExternalInput"))
    wc1_d = nc.dram_tensor("wc1", [F, F], my_dt, kind="ExternalInput")
    bc1_d = nc.dram_tensor("bc1", [F, 1], f32, kind="ExternalInput")
    wc2_d = nc.dram_tensor("wc2", [F, NOUT], my_dt, kind="ExternalInput")
    bc2_d = nc.dram_tensor("bc2", [NOUT, 1], f32, kind="ExternalInput")
    out_d = nc.dram_tensor("out", [NG, NOUT], f32, kind="ExternalOutput")
    DEBUG = os.environ.get("GIN_DEBUG", "0") == "1"
    if DEBUG:
        dbg_t1 = nc.dram_tensor("dbg_t1", [N, F], my_dt, kind="ExternalOutput")
        dbg_t2 = nc.dram_tensor("dbg_t2", [N, F], my_dt, kind="ExternalOutput")
        dbg_z0 = nc.dram_tensor("dbg_z0", [TILE, T * TILE], my_dt, kind="ExternalOutput")
        dbg_z1 = nc.dram_tensor("dbg_z1", [TILE, T * TILE], my_dt, kind="ExternalOutput")
        dbg_z2 = nc.dram_tensor("dbg_z2", [TILE, T * TILE], my_dt, kind="ExternalOutput")
        dbg_gb = nc.dram_tensor("dbg_gb", [NG, F], f32, kind="ExternalOutput")
        dbg_st0 = nc.dram_tensor("dbg_st0", [F, 2], f32, kind="ExternalOutput")
        dbg_g = nc.dram_tensor("dbg_g", [NG, F], f32, kind="ExternalOutput")

    # ---- internal DRAM ----
    tables = [xt]
    shards = []
    stats_in, stats_out = [], []
    for l in range(L - 1):
        tables.append(nc.dram_tensor(f"table{l + 1}", [N, F], my_dt,
                                     addr_space="Shared"))
        shards.append(nc.dram_tensor(f"shard{l}", [NSHARD, F], my_dt))
    for l in range(L):
        stats_in.append(nc.dram_tensor(f"stats_in{l}", [F, 2], f32))
        stats_out.append(nc.dram_tensor(f"stats_out{l}", [F, 2], f32,
                                        addr_space="Shared"))
    gbuf = nc.dram_tensor("gbuf", [NG, F], f32)
    gar = nc.dram_tensor("gar", [NG, F], f32, addr_space="Shared")

    with tile.TileContext(nc) as tc, ExitStack() as ctx:
        const = ctx.enter_context(tc.tile_pool(name="const", bufs=1))
        meta = ctx.enter_context(tc.tile_pool(name="meta", bufs=1))
        gpool = ctx.enter_context(tc.tile_pool(name="gather", bufs=2))
        ipool = ctx.enter_context(tc.tile_pool(name="idxp", bufs=3))
        spool = ctx.enter_context(tc.tile_pool(name="sel", bufs=4))
        zpool = ctx.enter_context(tc.tile_pool(name="zbuf", bufs=1))
        stpool = ctx.enter_context(tc.tile_pool(name="stats", bufs=2))
        hpool = ctx.enter_context(tc.tile_pool(name="htiles", bufs=3))
        vpool = ctx.enter_context(tc.tile_pool(name="vecs", bufs=2))
        ppool_a = ctx.enter_context(tc.tile_pool(name="ps_aggr", bufs=2, space="PSUM"))
        ppool_z = ctx.enter_context(tc.tile_pool(name="ps_z", bufs=2, space="PSUM"))
        ppool_y = ctx.enter_context(tc.tile_pool(name="ps_y", bufs=2, space="PSUM"))
        ppool_t = ctx.enter_context(tc.tile_pool(name="ps_tr", bufs=1, space="PSUM"))

        # ---- constants ----
        ident = const.tile([TILE, TILE], my_dt)
        make_identity(nc, ident[:])
        ident_f = const.tile([TILE, TILE], f32)
        make_identity(nc, ident_f[:])
        iota_i = const.tile([TILE, GWIN], i32)
        nc.gpsimd.iota(iota_i[:], pattern=[[1, GWIN]], base=0, channel_multiplier=0)
        iota_bc = const.tile([TILE, GWIN], my_dt)
        nc.vector.tensor_copy(iota_bc[:], iota_i[:])

        # ---- resident metadata ----
        dloc_sb = meta.tile([TILE, QCOL], f32)
        nc.sync.dma_start(dloc_sb[:], dloc_d[:, :])
        bl_sb = meta.tile([TILE, T], f32)
        nc.sync.dma_start(bl_sb[:], bl_d[:, :])
        grows_sb = meta.tile([TILE, 2], i32)
        nc.sync.dma_start(grows_sb[:], grows_d[:, :])

        w1_sb, w2_sb, gam_sb, bet_sb, b2v_sb = [], [], [], [], []
        for l in range(L):
            w1 = meta.tile([F, F], my_dt, tag=f"w1_{l}")
            nc.sync.dma_start(w1[:], w1_d[l][:, :])
            w1_sb.append(w1)
            w2 = meta.tile([F, F], my_dt, tag=f"w2_{l}")
            nc.sync.dma_start(w2[:], w2_d[l][:, :])
            w2_sb.append(w2)
            g = meta.tile([F, 1], f32, tag=f"g_{l}")
            nc.sync.dma_start(g[:], gam_d[l][:, :])
            gam_sb.append(g)
            b = meta.tile([F, 1], f32, tag=f"b_{l}")
            nc.sync.dma_start(b[:], bet_d[l][:, :])
            bet_sb.append(b)
            b2 = meta.tile([F, 1], f32, tag=f"b2_{l}")
            nc.sync.dma_start(b2[:], b2_d[l][:, :])
            b2v_sb.append(b2)
        wc1_sb = meta.tile([F, F], my_dt)
        nc.sync.dma_start(wc1_sb[:], wc1_d[:, :])
        bc1_sb = meta.tile([F, 1], f32)
        nc.sync.dma_start(bc1_sb[:], bc1_d[:, :])
        wc2_sb = meta.tile([F, NOUT], my_dt)
        nc.sync.dma_start(wc2_sb[:], wc2_d[:, :])
        bc2_sb = meta.tile([NOUT, 1], f32)
        nc.sync.dma_start(bc2_sb[:], bc2_d[:, :])

        zero_t = const.tile([TILE, F], f32)
        nc.gpsimd.memset(zero_t[:], 0.0)
        eps_t = const.tile([F, 1], f32)
        nc.gpsimd.memset(eps_t[:], BN_EPS)

        pool_ps = None  # pooling psum tiles, created in layer L-1

        for l in range(L):
            table = tables[l]
            zbuf = zpool.tile([TILE, T * TILE], my_dt, tag="zbuf")
            szb = stpool.tile([F, T], f32, tag="sz")
            sz2b = stpool.tile([F, T], f32, tag="sz2")

            # ---------- phase A: gather + aggregate + z matmul + stats ----------
            for g in range(NGRP):
                itile = ipool.tile([TILE, 4 * (NI // 16)], i16, tag="idxt")
                nc.sync.dma_start(
                    itile[:], idx_d[:, g * 4 * (NI // 16):
                                    (g + 1) * 4 * (NI // 16)])
                gt = gpool.tile([TILE, GCOLS, TILE], my_dt, tag="G")
                for c4 in range(4):
                    nc.gpsimd.dma_gather(
                        out_ap=gt[:, c4 * GROUP * C4:(c4 + 1) * GROUP * C4, :],
                        in_ap=table[c4 * 25000:(c4 + 1) * 25000, :],
                        idxs_ap=itile[:, c4 * (NI // 16):
                                      (c4 + 1) * (NI // 16)],
                        num_idxs=NI, num_idxs_reg=NI, elem_size=F,
                        single_packet=False)
                for ti in range(GROUP):
                    t = g * GROUP + ti
                    pa = ppool_a.tile([TILE, TILE], f32, tag="aggr")
                    k = 0
                    for c4 in range(4):
                        for j in range(C4):
                            lq = (c4 * GROUP + ti) * C4 + j
                            q = g * GCOLS + lq
                            s = spool.tile([TILE, TILE], my_dt, tag="S")
                            nc.vector.tensor_scalar(
                                out=s[:], in0=iota_bc[:, 0:TILE],
                                scalar1=dloc_sb[:, q:q + 1],
                                scalar2=None, op0=OP.is_equal)
                            nc.tensor.matmul(
                                pa[:], lhsT=gt[:, lq, :], rhs=s[:],
                                start=(k == 0), stop=(k == 4 * C4 - 1))
                            k += 1
                    az = spool.tile([TILE, TILE], my_dt, tag="aggrS")
                    nc.vector.tensor_copy(az[:], pa[:])
                    pz = ppool_z.tile([TILE, TILE], f32, tag="z")
                    nc.tensor.matmul(pz[:], lhsT=w1_sb[l][:], rhs=az[:],
                                     start=True, stop=True)
                    nc.scalar.activation(
                        out=zbuf[:, t * TILE:(t + 1) * TILE], in_=pz[:],
                        func=AF.Copy, accum_out=szb[:, t:t + 1])
                    sq = spool.tile([TILE, TILE], my_dt, tag="sq")
                    nc.scalar.activation(out=sq[:], in_=pz[:], func=AF.Square,
                                         accum_out=sz2b[:, t:t + 1])

            # ---------- BN stats AllReduce ----------
            stv = stpool.tile([F, 2], f32, tag="statv")
            nc.vector.tensor_reduce(stv[:, 0:1], szb[:], axis=AX.X, op=OP.add)
            nc.vector.tensor_reduce(stv[:, 1:2], sz2b[:], axis=AX.X, op=OP.add)
            nc.sync.dma_start(stats_in[l][:, :], stv[:])
            nc.gpsimd.collective_compute(
                "AllReduce", OP.add, replica_groups=rg,
                ins=[stats_in[l][:, :]], outs=[stats_out[l][:, :]])
            star = vpool.tile([F, 2], f32, tag="star")
            nc.sync.dma_start(star[:], stats_out[l][:, :])

            mu = vpool.tile([F, 1], f32, tag="mu")
            nc.vector.tensor_scalar(out=mu[:], in0=star[:, 0:1], scalar1=1.0 / N,
                                    scalar2=None, op0=OP.mult)
            e2 = vpool.tile([F, 1], f32, tag="e2")
            nc.vector.tensor_scalar(out=e2[:], in0=star[:, 1:2], scalar1=1.0 / N,
                                    scalar2=None, op0=OP.mult)
            musq = vpool.tile([F, 1], f32, tag="musq")
            nc.vector.tensor_tensor(out=musq[:], in0=mu[:], in1=mu[:], op=OP.mult)
            var = vpool.tile([F, 1], f32, tag="var")
            nc.vector.tensor_tensor(out=var[:], in0=e2[:], in1=musq[:],
                                    op=OP.subtract)
            std = vpool.tile([F, 1], f32, tag="std")
            nc.scalar.activation(out=std[:], in_=var[:], func=AF.Sqrt,
                                 bias=eps_t[:])
            rstd = vpool.tile([F, 1], f32, tag="rstd")
            nc.vector.reciprocal(rstd[:], std[:])
            Ati = vpool.tile([F, 1], f32, tag="Ati")
            nc.vector.tensor_tensor(out=Ati[:], in0=rstd[:], in1=gam_sb[l][:],
                                    op=OP.mult)
            mB = vpool.tile([F, 1], f32, tag="mB")
            nc.vector.tensor_tensor(out=mB[:], in0=mu[:], in1=Ati[:], op=OP.mult)
            Bti = vpool.tile([F, 1], f32, tag="Bti")
            nc.vector.tensor_tensor(out=Bti[:], in0=bet_sb[l][:], in1=mB[:],
                                    op=OP.subtract)

            # ---------- phase B: normalize + W2 + relu (+store / +pool) ----------
            if l == L - 1:
                gps0 = ppool_a.tile([TILE, TILE], f32, tag="aggr")
                gps1 = ppool_a.tile([TILE, TILE], f32, tag="aggr")
                pool_ps = [gps0[:], gps1[:]]
            for t in range(T):
                ht = hpool.tile([TILE, TILE], my_dt, tag="hrelu")
                nc.scalar.activation(out=ht[:], in_=zbuf[:, t * TILE:(t + 1) * TILE],
                                     func=AF.Relu, bias=Bti[:], scale=Ati[:])
                py = ppool_y.tile([TILE, TILE], f32, tag="y")
                nc.tensor.matmul(py[:], lhsT=w2_sb[l][:], rhs=ht[:],
                                 start=True, stop=True)
                ho = hpool.tile([TILE, TILE], my_dt, tag="hout")
                nc.scalar.activation(out=ho[:], in_=py[:], func=AF.Relu,
                                     bias=b2v_sb[l][:], scale=1.0)
                pt = ppool_t.tile([TILE, TILE], my_dt, tag="tr")
                nc.tensor.transpose(pt[:], ho[:], ident[:])
                hn = hpool.tile([TILE, TILE], my_dt, tag="hnode")
                nc.vector.tensor_copy(hn[:], pt[:])
                rows = LAST_ROWS if t == T - 1 else TILE
                if l < L - 1:
                    nc.sync.dma_start(
                        out=shards[l][t * TILE:t * TILE + rows, :],
                        in_=hn[:rows, :])
                else:
                    for h in range(2):
                        bs = spool.tile([TILE, TILE], my_dt, tag="bsel")
                        nc.vector.tensor_scalar(
                            out=bs[:], in0=iota_bc[:, h * TILE:(h + 1) * TILE],
                            scalar1=bl_sb[:, t:t + 1], scalar2=None,
                            op0=OP.is_equal)
                        nc.tensor.matmul(pool_ps[h], lhsT=bs[:], rhs=hn[:],
                                         start=(t == 0), stop=(t == T - 1),
                                         skip_group_check=True)

            if l < L - 1:
                nc.gpsimd.collective_compute(
                    "AllGather", OP.bypass, replica_groups=rg,
                    ins=[shards[l][:, :]], outs=[tables[l + 1][:, :]])
            if DEBUG and l == 0:
                nc.sync.dma_start(dbg_z0[:, :], zbuf[:])
                nc.sync.dma_start(dbg_st0[:, :], stats_in[l][:, :])
                nc.sync.dma_start(dbg_t1[:, :], tables[1][:, :])
            if DEBUG and l == 1:
                nc.sync.dma_start(dbg_t2[:, :], tables[2][:, :])
                nc.sync.dma_start(dbg_z1[:, :], zbuf[:])
            if DEBUG and l == 2:
                nc.sync.dma_start(dbg_z2[:, :], zbuf[:])

        # ---------- pooling scatter + AllReduce ----------
        for b in range(NG // TILE):
            nc.sync.dma_start(gbuf[b * TILE:(b + 1) * TILE, :], zero_t[:])
        for h in range(2):
            pgs = hpool.tile([TILE, F], f32, tag="gpart")
            nc.vector.tensor_copy(pgs[:], pool_ps[h])
            nc.gpsimd.indirect_dma_start(
                out=gbuf[:, :],
                out_offset=bass.IndirectOffsetOnAxis(ap=grows_sb[:, h:h + 1],
                                                     axis=0),
                in_=pgs[:],
                in_offset=None,
                bounds_check=NG - 1,
                oob_is_err=False,
            )
        nc.gpsimd.collective_compute(
            "AllReduce", OP.add, replica_groups=rg,
            ins=[gbuf[:, :]], outs=[gar[:, :]])
        if DEBUG:
            nc.sync.dma_start(dbg_g[:, :], gar[:, :])
            nc.sync.dma_start(dbg_gb[:, :], gbuf[:, :])

        # ---------- classifier (replicated over all graph blocks) ----------
        for b in range(NG // TILE):
            gt = hpool.tile([TILE, F], f32, tag="gtile")
            nc.sync.dma_start(gt[:], gar[b * TILE:(b + 1) * TILE, :])
            ptr = ppool_a.tile([TILE, TILE], f32, tag="aggr")
            nc.tensor.transpose(ptr[:], gt[:], ident_f[:])
            gT = hpool.tile([TILE, TILE], my_dt, tag="gT")
            nc.vector.tensor_copy(gT[:], ptr[:])
            p1 = ppool_z.tile([TILE, TILE], f32, tag="z")
            nc.tensor.matmul(p1[:], lhsT=wc1_sb[:], rhs=gT[:], start=True,
                             stop=True)
            g1 = hpool.tile([TILE, TILE], my_dt, tag="g1")
            nc.scalar.activation(out=g1[:], in_=p1[:], func=AF.Relu,
                                 bias=bc1_sb[:], scale=1.0)
            p2 = ppool_y.tile([NOUT, TILE], f32, tag="y")
            nc.tensor.matmul(p2[:], lhsT=wc2_sb[:], rhs=g1[:], start=True,
                             stop=True)
            o2 = hpool.tile([NOUT, TILE], f32, tag="o2")
            nc.vector.tensor_scalar(out=o2[:], in0=p2[:], scalar1=bc2_sb[:, 0:1],
                                    scalar2=None, op0=OP.add)
            pf = ppool_a.tile([TILE, NOUT], f32, tag="aggr")
            nc.tensor.transpose(pf[:], o2[:], ident_f[:NOUT, :NOUT])
            of = hpool.tile([TILE, NOUT], f32, tag="ofs")
            nc.vector.tensor_copy(of[:], pf[:])
            nc.sync.dma_start(out_d[b * TILE:(b + 1) * TILE, :], of[:])

    nc.compile()
    return nc


# ======================================================================
# entry point
# ======================================================================

_CACHE = {}
_LAST_IN_MAPS = None


def kernel(x, edge_index, batch, params):
    global _LAST_IN_MAPS
    from concourse import mybir
    from concourse.bass_utils import run_bass_kernel_spmd

    np_dt = {"float32": np.float32, "bfloat16": None}[DT_STR]
    if DT_STR == "bfloat16":
        import ml_dtypes
        np_dt = ml_dtypes.bfloat16
    my_dt = {"float32": mybir.dt.float32,
             "bfloat16": mybir.dt.bfloat16}[DT_STR]

    perm, idx16, dl, C4 = _prep_edges(edge_index)
    bl, grows = _prep_batch(batch, perm)

    key = (C4, DT_STR)
    if key not in _CACHE:
        _CACHE[key] = _build(C4, np_dt, my_dt)
    nc = _CACHE[key]

    x_t = np.ascontiguousarray(
        np.asarray(x, np.float32)[perm]).astype(np_dt)

    def _w(a):
        return np.ascontiguousarray(np.asarray(a, np.float32)).astype(np_dt)

    def _v(a, n):
        return np.ascontiguousarray(
            np.asarray(a, np.float32).reshape(n, 1))

    in_maps = []
    for c in range(NCORES):
        m = {
            "x_table": x_t,
            "idx": np.ascontiguousarray(idx16[c]),
            "dloc": np.ascontiguousarray(dl[c]),
            "blocal": np.ascontiguousarray(bl[c]),
            "grows": np.ascontiguousarray(grows[c]),
            "wc1": _w(params["cls"]["W1"]),
            "bc1": _v(params["cls"]["b1"], F),
            "wc2": _w(params["cls"]["W2"]),
            "bc2": _v(params["cls"]["b2"], NOUT),
        }
        for l in range(L):
            p = params["convs"][l]
            m[f"w1_{l}"] = _w(p["W1"])
            m[f"w2_{l}"] = _w(p["W2"])
            m[f"gam_{l}"] = _v(p["gamma"], F)
            m[f"bet_{l}"] = _v(p["beta"], F)
            m[f"b2_{l}"] = _v(p["b2"], F)
        in_maps.append(m)

    _LAST_IN_MAPS = in_maps
    res = run_bass_kernel_spmd(nc, in_maps, core_ids=list(range(NCORES)))
    global _LAST_RES
    _LAST_RES = res
    return np.asarray(res.results[0]["out"], np.float32)
